# revision 17
# baseline (speedup 1.0000x reference)
"""Trainium2 Bass kernel for batched general-score attention.

Reference computation (B=32, L=2048, H=2048):
    proj     = enc @ W^T + b          # [B, L, H]
    energies = proj . hidden          # [B, L]
    attn     = softmax(energies, 1)   # [B, L, 1]

Algebraic rewrite used here:
    energies = enc @ (W^T hidden) + (b . hidden)
The (b . hidden) term is constant across L for a batch, and softmax is
invariant to per-row constants, so it drops out entirely.  This collapses
the O(B*L*H^2) matmul into an O(B*H^2) matvec + O(B*L*H) batched dot.
The tiny matvec V = hidden @ W (134 MFLOP, 0.05% of the reference FLOPs)
is done host-side in fp32 BLAS while sharding the inputs.

Precision: enc and V are downcast to fp16 host-side.  Measured on the
fixed problem seed this gives rel_err 5.9e-3 (fp32 energies accumulate
on-chip; softmax in fp32), comfortably inside the 2e-2 gate, while
halving HBM traffic (the binding roofline: 32 MB/core at ~358 GB/s) and
doubling DVE multiply-reduce throughput (2x_1P perf mode for 16-bit).

Sharding: data-parallel over batch.  8 cores x 4 batches each.  Each core:
  1. broadcasts its 4 fp16 V rows across 128 partitions,
  2. streams its 32 MB enc slice in [128, 4, 2048] fp16 chunks laid out
     so partition p holds rows p*16+j of the batch (16 KB contiguous
     per partition per chunk); one fused scalar_tensor_tensor
     (mult + accumulated row-sum) per [128, 2048] tile produces the
     energy column -> energies land as [128, 16] fp32 per batch.
     Tiles are split 5:11 between a fused-STT lane (DVE only) and a
     tensor_tensor-multiply (DVE, 2x perf mode) + activation-Copy
     accumulate (ACT) lane so DVE and ACT share the reduction work,
  3. softmax per batch: DVE row-max, PE-transpose cross-partition max,
     rank-1 (-ones)-matmul broadcast of the max, ScalarE exp with
     accumulated row-sum, all-ones matmul for cross-partition sum (with
     broadcast), reciprocal, tensor_scalar multiply,
  4. DMAs the [128, 16] attention tile back; with l = p*16 + t each
     partition's 16 outputs are contiguous in DRAM.

Only stock-ISA instructions are used.
"""

import sys

if "/opt/trn_rl_repo" not in sys.path:
    sys.path.insert(0, "/opt/trn_rl_repo")

from contextlib import ExitStack

import numpy as np

import concourse.bacc as bacc
import concourse.bass as bass
import concourse.mybir as mybir
import concourse.tile as tile
from concourse._compat import with_exitstack
from concourse.bass_utils import run_bass_kernel_spmd

B, L, H = 32, 2048, 2048
N_CORES = 8
BL = B // N_CORES  # batches per core
P = 128            # partitions
LT = L // P        # L tiles per batch (= columns of the energy tile)
CH = 8             # L tiles per DMA chunk (4 MB fp16, 32 KB/partition runs)

F32 = mybir.dt.float32
F16 = mybir.dt.float16


@with_exitstack
def _attn_kernel(ctx: ExitStack, tc: tile.TileContext,
                 enc: bass.AP, v: bass.AP, out: bass.AP):
    nc = tc.nc

    singles = ctx.enter_context(tc.tile_pool(name="singles", bufs=1))
    encpool = ctx.enter_context(tc.tile_pool(name="encpool", bufs=4))
    vbpool = ctx.enter_context(tc.tile_pool(name="vbpool", bufs=BL))
    scratch = ctx.enter_context(tc.tile_pool(name="scratch", bufs=2))
    tprod = ctx.enter_context(tc.tile_pool(name="tprod", bufs=3))
    actjunk = ctx.enter_context(tc.tile_pool(name="actjunk", bufs=2))
    small = ctx.enter_context(tc.tile_pool(name="small", bufs=4 * BL))
    psum = ctx.enter_context(tc.tile_pool(name="psum", bufs=2, space="PSUM"))

    neg_ones_row = singles.tile([1, P], F32)
    nc.vector.memset(neg_ones_row, -1.0)
    ones_sq = singles.tile([P, P], F32)
    nc.vector.memset(ones_sq, 1.0)
    # identity for the PE-transpose in softmax
    ident_dram = nc.inline_tensor(np.eye(P, dtype=np.float32), name="ident")
    ident = singles.tile([P, P], F32)
    nc.gpsimd.dma_start(out=ident, in_=ident_dram.ap())

    # Warm the exp table while DMAs stream.
    warm = singles.tile([1, 1], F32)
    nc.vector.memset(warm, 0.0)
    nc.scalar.activation(warm, warm, mybir.ActivationFunctionType.Exp)

    # ---- V rows arrive host-prebroadcast as [BL*128, H] fp16.
    # Non-enc DMAs mostly ride the GpSimd SWDGE ring: the ACT engine
    # does accumulation compute, so a dma_start in its HWDGE FIFO would
    # queue behind ~2 us ops and starve the stream.  vb[0] gates the
    # very first STT, and the SWDGE path pays a ~10 us Q7 cold-start —
    # so vb[0] goes FIRST on the SyncE HWDGE ring, ahead of enc chunk 0;
    # vb[1..3] are deferred into the enc stream (below).
    vb = []
    for _i in range(BL):
        vb_b = vbpool.tile([P, H], F16, tag="vb_b")
        vb.append(vb_b)
    nc.sync.dma_start(out=vb[0], in_=v[0:P, :])

    # ---- stream enc chunks; fused multiply+reduce -> energies ----
    # The softmax of batch b-1 is software-pipelined into batch b's STT
    # stream: DVE runs its instruction stream in program order, so an
    # un-pipelined softmax stalls DVE on the cross-engine chain at every
    # batch boundary (and the stalled consumer backs up the enc DMA ring).
    def softmax_steps(b, e_b):
        # step 0
        m_p = small.tile([P, 1], F32, tag="m")
        nc.vector.reduce_max(m_p, e_b, axis=mybir.AxisListType.X)
        # cross-partition max: PE transpose [128,1]->[1,128], reduce free
        mt_ps = psum.tile([1, P], F32, tag="ps")
        nc.tensor.transpose(mt_ps, m_p, ident)
        yield
        # step 1
        m_s = small.tile([1, 1], F32, tag="ms")
        nc.vector.reduce_max(m_s, mt_ps, axis=mybir.AxisListType.X)
        # broadcast -max to all partitions: (-ones)[1,128].T @ max[1,1]
        negm_ps = psum.tile([P, 1], F32, tag="ps")
        nc.tensor.matmul(negm_ps, lhsT=neg_ones_row, rhs=m_s,
                         start=True, stop=True)
        neg_m = small.tile([P, 1], F32, tag="negm")
        nc.scalar.copy(neg_m, negm_ps)
        yield
        # step 2
        p_un = small.tile([P, LT], F32, tag="p")
        s_p = small.tile([P, 1], F32, tag="s")
        nc.scalar.activation(
            p_un, e_b, mybir.ActivationFunctionType.Exp,
            bias=neg_m[:, 0:1], accum_out=s_p)
        yield
        # step 3: sum across partitions AND broadcast in one matmul:
        # ones[128,128].T @ s_p[128,1] -> [128,1] all-partitions total
        s_ps = psum.tile([P, 1], F32, tag="ps")
        nc.tensor.matmul(s_ps, lhsT=ones_sq, rhs=s_p, start=True, stop=True)
        s_all = small.tile([P, 1], F32, tag="sall")
        nc.scalar.copy(s_all, s_ps)
        yield
        # step 4
        r_p = small.tile([P, 1], F32, tag="r")
        nc.vector.reciprocal(r_p, s_all)
        yield
        # step 5
        attn = small.tile([P, LT], F32, tag="attn")
        nc.vector.tensor_scalar_mul(attn, p_un, r_p[:, 0:1])
        yield
        # step 6: out[b, p*16 + t] = attn[p, t] — contiguous 64 B per
        # partition.  On the SWDGE ring: in the SyncE FIFO it would
        # head-of-line block the enc stream, and on ScalarE it would
        # queue behind the accumulation compute.
        nc.gpsimd.dma_start(
            out=out.rearrange("bl (p t) -> bl p t", p=P)[b],
            in_=attn,
        )
        yield

    pending = None
    chunk_idx = 0
    for b in range(BL):
        e_b = small.tile([P, LT], F32, tag="e")
        # partition p holds batch rows p*16 + t; chunk (t0, n) covers
        # columns t0..t0+n-1 -> per-partition contiguous n*4 KB DRAM run
        enc_b = enc[b * L:(b + 1) * L, :].rearrange("(p n) h -> p n h", p=P)
        if b == 0:
            # small tiles first so DVE starts sooner, then 4 MB chunks
            plan = [(0, 1), (1, 1), (2, 2), (4, 4), (8, 8)]
        else:
            plan = [(0, 8), (8, 8)]
        for t_start, ntile in plan:
            enc_t = encpool.tile([P, CH, H], F16)
            # alternate full-size chunks between the SyncE HWDGE ring
            # and the (otherwise idle) GpSimd SWDGE ring: two queues
            # keep more descriptors in flight per SDMA engine; ramp
            # chunks stay on SyncE (SWDGE Q7 cold-start is ~10 us)
            ring = nc.gpsimd if (ntile == CH and chunk_idx % 2 == 0) else nc.sync
            chunk_idx += 1
            ring.dma_start(
                out=enc_t[:, 0:ntile, :],
                in_=enc_b[:, t_start:t_start + ntile, :])
            # deferred vb loads, well ahead of their first use at b=1..3
            if b == 0 and t_start in (2, 4, 8):
                vbi = {2: 1, 4: 2, 8: 3}[t_start]
                nc.gpsimd.dma_start(out=vb[vbi], in_=v[vbi * P:(vbi + 1) * P, :])
            for half in range(ntile):
                t = t_start + half
                # Lane split: the fused STT has no 2x DVE uop (2194 ns
                # per [128, 2048] fp16 tile, same as fp32), so only 5 of
                # 16 tiles use it.  The other 11 run the multiply as a
                # 2x-perf-mode tensor_tensor (1127 ns) and hand the
                # row-sum to the Scalar engine as an activation-Copy
                # with accumulate (2079 ns).  DVE and ACT then both
                # carry ~93 us/core, at the 32 MB / 358 GB/s DMA
                # roofline instead of DVE alone carrying 140 us.
                if t % 3 == 2:  # tiles 2,5,8,11,14: fused STT on DVE
                    prod = scratch.tile([P, H], F16)
                    nc.vector.scalar_tensor_tensor(
                        out=prod, in0=enc_t[:, half, :], scalar=1.0,
                        in1=vb[b], op0=mybir.AluOpType.mult,
                        op1=mybir.AluOpType.mult,
                        accum_out=e_b[:, t:t + 1])
                else:           # 11 tiles: TT on DVE + accum on ACT
                    prod = tprod.tile([P, H], F16)
                    nc.vector.tensor_tensor(
                        out=prod, in0=enc_t[:, half, :], in1=vb[b],
                        op=mybir.AluOpType.mult)
                    junk = actjunk.tile([P, H], F16)
                    nc.scalar.activation(
                        junk, prod, mybir.ActivationFunctionType.Copy,
                        accum_out=e_b[:, t:t + 1])
                if pending is not None and t >= 1:
                    next(pending, None)
        pending = softmax_steps(b, e_b)
    for _ in pending:
        pass


def build_program():
    nc = bacc.Bacc("TRN2", target_bir_lowering=False, debug=False,
                   enable_asserts=False, num_devices=N_CORES)
    enc = nc.dram_tensor("enc", [BL * L, H], F16, kind="ExternalInput")
    v = nc.dram_tensor("v", [BL * P, H], F16, kind="ExternalInput")
    out = nc.dram_tensor("out", [BL, L], F32, kind="ExternalOutput")
    with tile.TileContext(nc) as tc:
        _attn_kernel(tc, enc.ap(), v.ap(), out.ap())
    nc.compile()
    return nc


_NC_CACHE = {}


def _get_program():
    if "nc" not in _NC_CACHE:
        _NC_CACHE["nc"] = build_program()
    return _NC_CACHE["nc"]


def make_in_maps(hidden, encoder_outputs, W):
    hidden = np.asarray(hidden, dtype=np.float32)
    encoder_outputs = np.asarray(encoder_outputs)
    W = np.asarray(W, dtype=np.float32)
    V = (hidden[:, 0, :] @ W).astype(np.float16)  # [B, H] fp32 BLAS -> fp16
    # pre-broadcast each V row across the 128 partitions it will occupy
    Vb = np.ascontiguousarray(
        np.broadcast_to(V[:, None, :], (B, P, H)))  # [B, 128, H] fp16
    enc16 = encoder_outputs.astype(np.float16)
    in_maps = []
    for c in range(N_CORES):
        b0 = c * BL
        enc_c = np.ascontiguousarray(
            enc16[b0:b0 + BL].reshape(BL * L, H))
        in_maps.append({"enc": enc_c, "v": Vb[b0:b0 + BL].reshape(BL * P, H)})
    return in_maps


def kernel(hidden, encoder_outputs, W, b, **_):
    nc = _get_program()
    in_maps = make_in_maps(hidden, encoder_outputs, W)
    res = run_bass_kernel_spmd(nc, in_maps, core_ids=list(range(N_CORES)))
    out = np.concatenate(
        [res.results[c]["out"].reshape(BL, L, 1) for c in range(N_CORES)],
        axis=0)
    return out.astype(np.float32)


# revision 20
# speedup vs baseline: 1.3574x; 1.3574x over previous
"""Trainium2 Bass kernel for batched general-score attention.

Reference computation (B=32, L=2048, H=2048):
    proj     = enc @ W^T + b          # [B, L, H]
    energies = proj . hidden          # [B, L]
    attn     = softmax(energies, 1)   # [B, L, 1]

Algebraic rewrite used here:
    energies = enc @ (W^T hidden) + (b . hidden)
The (b . hidden) term is constant across L for a batch, and softmax is
invariant to per-row constants, so it drops out entirely.  This collapses
the O(B*L*H^2) matmul into an O(B*H^2) matvec + O(B*L*H) batched dot.
The tiny matvec V = hidden @ W (134 MFLOP, 0.05% of the reference FLOPs)
is done host-side in fp32 BLAS while sharding the inputs.

Precision: enc and V are downcast to fp16 host-side.  Measured on the
fixed problem seed this gives rel_err 5.9e-3 (fp32 energies accumulate
on-chip; softmax in fp32), comfortably inside the 2e-2 gate, while
halving HBM traffic (the binding roofline: 32 MB/core at ~358 GB/s) and
doubling DVE multiply-reduce throughput (2x_1P perf mode for 16-bit).

Sharding: data-parallel over batch.  8 cores x 4 batches each.  Each core:
  1. broadcasts its 4 fp16 V rows across 128 partitions,
  2. streams its 32 MB enc slice in [128, 4, 2048] fp16 chunks laid out
     so partition p holds rows p*16+j of the batch (16 KB contiguous
     per partition per chunk); one fused scalar_tensor_tensor
     (mult + accumulated row-sum) per [128, 2048] tile produces the
     energy column -> energies land as [128, 16] fp32 per batch.
     Tiles are split 5:11 between a fused-STT lane (DVE only) and a
     tensor_tensor-multiply (DVE, 2x perf mode) + activation-Copy
     accumulate (ACT) lane so DVE and ACT share the reduction work,
  3. softmax per batch: DVE row-max, PE-transpose cross-partition max,
     rank-1 (-ones)-matmul broadcast of the max, ScalarE exp with
     accumulated row-sum, all-ones matmul for cross-partition sum (with
     broadcast), reciprocal, tensor_scalar multiply,
  4. DMAs the [128, 16] attention tile back; with l = p*16 + t each
     partition's 16 outputs are contiguous in DRAM.

Only stock-ISA instructions are used.
"""

import sys

if "/opt/trn_rl_repo" not in sys.path:
    sys.path.insert(0, "/opt/trn_rl_repo")

from contextlib import ExitStack

import numpy as np

import concourse.bacc as bacc
import concourse.bass as bass
import concourse.mybir as mybir
import concourse.tile as tile
from concourse._compat import with_exitstack
from concourse.bass_utils import run_bass_kernel_spmd

B, L, H = 32, 2048, 2048
N_CORES = 8
BL = B // N_CORES  # batches per core
P = 128            # partitions
LT = L // P        # L tiles per batch (= columns of the energy tile)
CH = 4             # L tiles per DMA chunk (2 MB fp16, 16 KB/partition runs)

F32 = mybir.dt.float32
F16 = mybir.dt.float16


@with_exitstack
def _attn_kernel(ctx: ExitStack, tc: tile.TileContext,
                 enc: bass.AP, v: bass.AP, out: bass.AP):
    nc = tc.nc

    singles = ctx.enter_context(tc.tile_pool(name="singles", bufs=1))
    encpool = ctx.enter_context(tc.tile_pool(name="encpool", bufs=8))
    vbpool = ctx.enter_context(tc.tile_pool(name="vbpool", bufs=BL))
    scratch = ctx.enter_context(tc.tile_pool(name="scratch", bufs=2))
    tprod = ctx.enter_context(tc.tile_pool(name="tprod", bufs=3))
    actjunk = ctx.enter_context(tc.tile_pool(name="actjunk", bufs=2))
    small = ctx.enter_context(tc.tile_pool(name="small", bufs=4 * BL))
    psum = ctx.enter_context(tc.tile_pool(name="psum", bufs=2, space="PSUM"))

    neg_ones_row = singles.tile([1, P], F32)
    nc.vector.memset(neg_ones_row, -1.0)
    ones_sq = singles.tile([P, P], F32)
    nc.vector.memset(ones_sq, 1.0)
    # identity for the PE-transpose in softmax
    ident_dram = nc.inline_tensor(np.eye(P, dtype=np.float32), name="ident")
    ident = singles.tile([P, P], F32)
    nc.gpsimd.dma_start(out=ident, in_=ident_dram.ap())

    # Warm the exp table while DMAs stream.
    warm = singles.tile([1, 1], F32)
    nc.vector.memset(warm, 0.0)
    nc.scalar.activation(warm, warm, mybir.ActivationFunctionType.Exp)

    # ---- V rows arrive host-prebroadcast as [BL*128, H] fp16.
    # Non-enc DMAs mostly ride the GpSimd SWDGE ring: the ACT engine
    # does accumulation compute, so a dma_start in its HWDGE FIFO would
    # queue behind ~2 us ops and starve the stream.  vb[0] gates the
    # very first STT, and the SWDGE path pays a ~10 us Q7 cold-start —
    # so vb[0] goes FIRST on the SyncE HWDGE ring, ahead of enc chunk 0;
    # vb[1..3] are deferred into the enc stream (below).
    vb = []
    for _i in range(BL):
        vb_b = vbpool.tile([P, H], F16, tag="vb_b")
        vb.append(vb_b)
    nc.sync.dma_start(out=vb[0], in_=v[0:P, :])

    # ---- stream enc chunks; fused multiply+reduce -> energies ----
    # The softmax of batch b-1 is software-pipelined into batch b's STT
    # stream: DVE runs its instruction stream in program order, so an
    # un-pipelined softmax stalls DVE on the cross-engine chain at every
    # batch boundary (and the stalled consumer backs up the enc DMA ring).
    def softmax_steps(b, e_b):
        # step 0
        m_p = small.tile([P, 1], F32, tag="m")
        nc.vector.reduce_max(m_p, e_b, axis=mybir.AxisListType.X)
        # cross-partition max: PE transpose [128,1]->[1,128], reduce free
        mt_ps = psum.tile([1, P], F32, tag="ps")
        nc.tensor.transpose(mt_ps, m_p, ident)
        yield
        # step 1
        m_s = small.tile([1, 1], F32, tag="ms")
        nc.vector.reduce_max(m_s, mt_ps, axis=mybir.AxisListType.X)
        # broadcast -max to all partitions: (-ones)[1,128].T @ max[1,1]
        negm_ps = psum.tile([P, 1], F32, tag="ps")
        nc.tensor.matmul(negm_ps, lhsT=neg_ones_row, rhs=m_s,
                         start=True, stop=True)
        neg_m = small.tile([P, 1], F32, tag="negm")
        nc.scalar.copy(neg_m, negm_ps)
        yield
        # step 2
        p_un = small.tile([P, LT], F32, tag="p")
        s_p = small.tile([P, 1], F32, tag="s")
        nc.scalar.activation(
            p_un, e_b, mybir.ActivationFunctionType.Exp,
            bias=neg_m[:, 0:1], accum_out=s_p)
        yield
        # step 3: sum across partitions AND broadcast in one matmul:
        # ones[128,128].T @ s_p[128,1] -> [128,1] all-partitions total
        s_ps = psum.tile([P, 1], F32, tag="ps")
        nc.tensor.matmul(s_ps, lhsT=ones_sq, rhs=s_p, start=True, stop=True)
        s_all = small.tile([P, 1], F32, tag="sall")
        nc.scalar.copy(s_all, s_ps)
        yield
        # step 4
        r_p = small.tile([P, 1], F32, tag="r")
        nc.vector.reciprocal(r_p, s_all)
        yield
        # step 5
        attn = small.tile([P, LT], F32, tag="attn")
        nc.vector.tensor_scalar_mul(attn, p_un, r_p[:, 0:1])
        yield
        # step 6: out[b, p*16 + t] = attn[p, t] — contiguous 64 B per
        # partition.  On the SWDGE ring: in the SyncE FIFO it would
        # head-of-line block the enc stream, and on ScalarE it would
        # queue behind the accumulation compute.
        nc.gpsimd.dma_start(
            out=out.rearrange("bl (p t) -> bl p t", p=P)[b],
            in_=attn,
        )
        yield

    pending = None
    chunk_idx = 0
    for b in range(BL):
        e_b = small.tile([P, LT], F32, tag="e")
        # partition p holds batch rows p*16 + t; chunk (t0, n) covers
        # columns t0..t0+n-1 -> per-partition contiguous n*4 KB DRAM run
        enc_b = enc[b * L:(b + 1) * L, :].rearrange("(p n) h -> p n h", p=P)
        if b == 0:
            # small tiles first so DVE starts sooner, then 2 MB chunks
            plan = [(0, 1), (1, 1), (2, 2), (4, 4), (8, 4), (12, 4)]
        else:
            plan = [(0, 4), (4, 4), (8, 4), (12, 4)]
        for t_start, ntile in plan:
            enc_t = encpool.tile([P, CH, H], F16)
            # the whole enc stream stays on the SyncE HWDGE ring: SWDGE
            # (GpSimd) moves bulk data measurably slower, and the ScalarE
            # HWDGE FIFO would queue behind ACT's ~2 us accum ops
            chunk_idx += 1
            nc.sync.dma_start(
                out=enc_t[:, 0:ntile, :],
                in_=enc_b[:, t_start:t_start + ntile, :])
            # deferred vb loads, well ahead of their first use at b=1..3
            if b == 0 and t_start in (2, 4, 8):
                vbi = {2: 1, 4: 2, 8: 3}[t_start]
                nc.gpsimd.dma_start(out=vb[vbi], in_=v[vbi * P:(vbi + 1) * P, :])
            for half in range(ntile):
                t = t_start + half
                # Lane split: the fused STT has no 2x DVE uop (2194 ns
                # per [128, 2048] fp16 tile, same as fp32), so only 5 of
                # 16 tiles use it.  The other 11 run the multiply as a
                # 2x-perf-mode tensor_tensor (1127 ns) and hand the
                # row-sum to the Scalar engine as an activation-Copy
                # with accumulate (2079 ns).  DVE and ACT then both
                # carry ~93 us/core, at the 32 MB / 358 GB/s DMA
                # roofline instead of DVE alone carrying 140 us.
                if t % 3 == 2:  # tiles 2,5,8,11,14: fused STT on DVE
                    prod = scratch.tile([P, H], F16)
                    nc.vector.scalar_tensor_tensor(
                        out=prod, in0=enc_t[:, half, :], scalar=1.0,
                        in1=vb[b], op0=mybir.AluOpType.mult,
                        op1=mybir.AluOpType.mult,
                        accum_out=e_b[:, t:t + 1])
                else:           # 11 tiles: TT on DVE + accum on ACT
                    prod = tprod.tile([P, H], F16)
                    nc.vector.tensor_tensor(
                        out=prod, in0=enc_t[:, half, :], in1=vb[b],
                        op=mybir.AluOpType.mult)
                    junk = actjunk.tile([P, H], F16)
                    nc.scalar.activation(
                        junk, prod, mybir.ActivationFunctionType.Copy,
                        accum_out=e_b[:, t:t + 1])
                if pending is not None and t >= 1:
                    next(pending, None)
        pending = softmax_steps(b, e_b)
    for _ in pending:
        pass


def build_program():
    nc = bacc.Bacc("TRN2", target_bir_lowering=False, debug=False,
                   enable_asserts=False, num_devices=N_CORES)
    enc = nc.dram_tensor("enc", [BL * L, H], F16, kind="ExternalInput")
    v = nc.dram_tensor("v", [BL * P, H], F16, kind="ExternalInput")
    out = nc.dram_tensor("out", [BL, L], F32, kind="ExternalOutput")
    with tile.TileContext(nc) as tc:
        _attn_kernel(tc, enc.ap(), v.ap(), out.ap())
    nc.compile()
    return nc


_NC_CACHE = {}


def _get_program():
    if "nc" not in _NC_CACHE:
        _NC_CACHE["nc"] = build_program()
    return _NC_CACHE["nc"]


def make_in_maps(hidden, encoder_outputs, W):
    hidden = np.asarray(hidden, dtype=np.float32)
    encoder_outputs = np.asarray(encoder_outputs)
    W = np.asarray(W, dtype=np.float32)
    V = (hidden[:, 0, :] @ W).astype(np.float16)  # [B, H] fp32 BLAS -> fp16
    # pre-broadcast each V row across the 128 partitions it will occupy
    Vb = np.ascontiguousarray(
        np.broadcast_to(V[:, None, :], (B, P, H)))  # [B, 128, H] fp16
    enc16 = encoder_outputs.astype(np.float16)
    in_maps = []
    for c in range(N_CORES):
        b0 = c * BL
        enc_c = np.ascontiguousarray(
            enc16[b0:b0 + BL].reshape(BL * L, H))
        in_maps.append({"enc": enc_c, "v": Vb[b0:b0 + BL].reshape(BL * P, H)})
    return in_maps


def kernel(hidden, encoder_outputs, W, b, **_):
    nc = _get_program()
    in_maps = make_in_maps(hidden, encoder_outputs, W)
    res = run_bass_kernel_spmd(nc, in_maps, core_ids=list(range(N_CORES)))
    out = np.concatenate(
        [res.results[c]["out"].reshape(BL, L, 1) for c in range(N_CORES)],
        axis=0)
    return out.astype(np.float32)
